# revision 16
# baseline (speedup 1.0000x reference)
"""Multi-head attention kernel for 8 Trainium2 NeuronCores.

Problem: O = softmax(Q @ K^T / sqrt(D)) @ V with B=8, H=12, N=1024, D=64, fp32.

Sharding: batch dim across the 8 cores (12 heads per core) — attention is
embarrassingly parallel over (b, h).

Device-side layout (host prep is free — only HW exec time counts):
  - Q, K are pre-transposed on host to [D, N] so the d-contraction of
    S = Q @ K^T has d on SBUF partitions for both operands.
  - S is computed *transposed* (S^T[k, q], k on partitions) so that the
    second matmul O^T = (V | 1)^T @ P^T needs no on-chip transposes at all.
  - Q^T is duplicated onto both partition halves and K^T chunks are packed
    in (even, odd) pairs on partition halves 0-63 / 64-127: the two K=64
    matmuls of a pair auto-derive tile_position (0,0)/(64,0) and run
    concurrently in the two row-halves of the PE array.
  - exp() is split across TWO engines so ScalarE stops being the pacing
    engine (exp is 98304 elem/lane; ScalarE alone = 81.9us @ 1.2GHz):
      * ScalarE: exact exp out of PSUM with the 1/sqrt(D) scale folded in.
      * VectorE (DVE): Schraudolph fp16 exp — one tensor_scalar
        (i16 = rint(s*A + B), A = 1024*log2(e)*scale, B = 15360 - 59) whose
        int16 bit pattern IS the fp16 approximation of exp(s*scale); the
        O matmul reads the tile bitcast to fp16. The -59 offset centers the
        piecewise-linear ripple so mixed exact/approx softmax rows carry no
        systematic bias (measured end-to-end rel err ~1.1e-2 at the default
        3/8 DVE share, vs the 2e-2 gate).
  - S^T chunk groups are pair-aligned (2 k-chunks = one concurrent PE pair
    per group, [128, 1024] fp32 = 2 PSUM banks, ring of 3) so every S
    matmul pair runs 2x on the PE array.
  - V gets a ones-column appended (65th weight column), so the P^T @ (V|1)
    matmul emits the softmax denominator as output row 64 for free.
  - The PE clock-gate (HAM) defaults to half rate; dummy warmup matmuls
    fill the initial input-DMA wait so the ramp starts as early as possible.
  - Normalization (divide by row 64) + final transpose happen on host.
"""

import os
from contextlib import ExitStack

import numpy as np

import concourse.tile as tile
from concourse import bacc, mybir
from concourse.bass_utils import run_bass_kernel_spmd

B, H, N, D = 8, 12, 1024, 64
NCORES = 8
HPC = B // NCORES * H  # heads per core = 12 (one full batch row per core)
KC = N // 128          # 8 key chunks of 128
JP = KC // 2           # 4 chunk pairs
QH = N // 512          # 2 query halves of 512

# Matmul dtype: fp16 streams at 1 cycle/row on the PE and halves DMA/SBUF.
_MM_DT = mybir.dt.float16

# Schraudolph bias offset (in 1/1024 fp16-exponent units): centers the
# piecewise-linear exp ripple so mixed exact/approx rows are unbiased.
_SCHR_OFF = float(os.environ.get("ATT_SCHR_OFF", "59"))

# Per-unit engine patterns for the 4 pair-groups (chunks 01|23|45|67).
# Alternating 1-dve / 2-dve units => 3/8 of exp on DVE. The global group
# sequence alternates engines (…sc,dve,sc,sc,dve,sc,dve,sc…) so the shared
# 3-deep PSUM ring is never drained by one engine's backlog.
_PAT_A = ("sc", "dve", "sc", "sc")
_PAT_B = ("dve", "sc", "dve", "sc")

LAST_RESULTS = None
_NC_CACHE = {}


def _install_ntff_hook():
    """Register the axon NTFF profile hook (the agent image's antenv lacks
    axon_hooks, so trn_boot degrades silently). Best-effort: tracing only."""
    import sys
    import types

    if "antenv.axon_hooks" in sys.modules:
        return
    try:
        import contextlib
        import ctypes

        so_path = "/opt/axon/libaxon_pjrt.so"
        lib = ctypes.CDLL(so_path)
        if not hasattr(lib, "axon_start_nrt_profile"):
            return
        lib.axon_start_nrt_profile.argtypes = [
            ctypes.POINTER(ctypes.c_int64),
            ctypes.c_size_t,
        ]
        lib.axon_start_nrt_profile.restype = ctypes.c_int64
        lib.axon_stop_nrt_profile.argtypes = [ctypes.c_char_p]
        lib.axon_stop_nrt_profile.restype = ctypes.c_int64

        @contextlib.contextmanager
        def _hook(output_dir, device_ids):
            import jax

            jax.devices()
            if device_ids:
                ids = (ctypes.c_int64 * len(device_ids))(*device_ids)
                rc = lib.axon_start_nrt_profile(ids, len(device_ids))
            else:
                rc = lib.axon_start_nrt_profile(None, 0)
            if rc != 0:
                raise RuntimeError(f"axon_start_nrt_profile rc={rc}")
            try:
                yield
            finally:
                n = lib.axon_stop_nrt_profile(str(output_dir).encode())
                print(f"ntff profile: {n} file(s) written to {output_dir}")

        mod = types.ModuleType("antenv.axon_hooks")
        mod.get_axon_ntff_profile_hook = lambda: _hook
        mod.set_axon_ntff_profile_hook = lambda h: None
        sys.modules["antenv.axon_hooks"] = mod
    except Exception:
        pass


def _emit(ctx, tc, qt, kt, vp, qk0a, qk0b, ot, mm_dt, scale):
    nc = tc.nc
    inp = ctx.enter_context(tc.tile_pool(name="inp", bufs=3))
    pts = ctx.enter_context(tc.tile_pool(name="pts", bufs=9))
    pti = ctx.enter_context(tc.tile_pool(name="pti", bufs=7))
    outp = ctx.enter_context(tc.tile_pool(name="outp", bufs=4))
    spsum = ctx.enter_context(tc.tile_pool(name="spsum", bufs=3, space="PSUM"))
    opsum = ctx.enter_context(tc.tile_pool(name="opsum", bufs=2, space="PSUM"))

    a_dve = float(1024.0 * np.log2(np.e) * scale)
    b_dve = float(15360.0 - _SCHR_OFF)

    def emit_loads(h):
        """Input DMA loads for head h. kt before qt (LDWEIGHTS consumes kt
        first); vp on the gpsimd queue — it is only needed by the O phase and
        must not delay the S loads."""
        if h == 0:
            # Head 0: K^T/Q^T arrive as host-packed DMAs split across
            # queues so the first S group's operands land with parallel
            # transfers — the head is DMA-latency bound. (Only SP issues
            # here: the Act ring is blocked by ACT_TABLE_LOAD until ~9us
            # and DVE cannot issue DMAs.)
            qk_a = inp.tile([128, 768], mm_dt, tag="qk0a", name="qk0a")
            nc.sync.dma_start(qk_a[:], qk0a[0])
            qk_b = inp.tile([128, 768], mm_dt, tag="qk0b", name="qk0b")
            # kt pairs 2-3 ride the fast SP ring right behind qk0a (unit 0's
            # g2/g3 are gated on them); qt q-half 1 is only needed by unit 1
            # and can take the slower SWDGE ring.
            nc.sync.dma_start(qk_b[:, 0:256], qk0b[0, :, 0:256])
            nc.gpsimd.dma_start(qk_b[:, 256:768], qk0b[0, :, 256:768])

            def kt_at(rows, jp):
                if jp < 2:
                    return qk_a[rows, jp * 128 : (jp + 1) * 128]
                return qk_b[rows, (jp - 2) * 128 : (jp - 1) * 128]

            def qt_at(rows, qh):
                src_t = qk_a if qh == 0 else qk_b
                return src_t[rows, 256:768]
        else:
            kt_t = inp.tile([128, JP * 128], mm_dt, tag="kt", name=f"kt{h}")
            nc.sync.dma_start(kt_t[:], kt[h])
            qt_t = inp.tile([128, N], mm_dt, tag="qt", name=f"qt{h}")
            nc.sync.dma_start(qt_t[:, 0:512], qt[h, :, 0:512])
            nc.sync.dma_start(qt_t[:, 512:1024], qt[h, :, 512:1024])

            def kt_at(rows, jp):
                return kt_t[rows, jp * 128 : (jp + 1) * 128]

            def qt_at(rows, qh):
                return qt_t[rows, qh * 512 : (qh + 1) * 512]

        vp_t = inp.tile([128, KC * 65], mm_dt, tag="vp", name=f"vp{h}")
        nc.gpsimd.dma_start(vp_t[:], vp[h])
        return kt_at, qt_at, vp_t

    def emit_o_mms(h, qh, vp_t, chunks, po):
        """O^T accumulation matmuls for a subset of k-chunks of one unit.
        O^T[m, q] = sum_k (V|1)[k, m] * P^T[k, q]; row 64 is the softmax
        denominator. `chunks` is [(kc, pt, off)] — each chunk's exp lives
        at column offset `off` of (possibly shared, cross-unit) pt tiles."""
        for kc, pt, off in chunks:
            rhs = pt[:, off : off + 512]
            if rhs.dtype == mybir.dt.int16:
                rhs = rhs.bitcast(mybir.dt.float16)
            nc.tensor.matmul(
                po[:],
                lhsT=vp_t[:, kc * 65 : (kc + 1) * 65],
                rhs=rhs,
                start=(kc == 0),
                stop=(kc == KC - 1),
            )

    def emit_o_out(h, qh, po, copy_eng="v", split_store=False):
        """PSUM->SBUF copy-out + store for one finished O^T accumulation."""
        qs = slice(qh * 512, (qh + 1) * 512)
        o_sb = outp.tile([65, 512], mybir.dt.float32, tag="osb", name=f"ou{h}_{qh}")
        if copy_eng == "v":
            nc.vector.tensor_copy(o_sb[:], po[:])
        elif copy_eng == "split":
            nc.vector.tensor_copy(o_sb[:, 0:256], po[:, 0:256])
            nc.scalar.copy(o_sb[:, 256:512], po[:, 256:512])
        else:
            nc.scalar.copy(o_sb[:], po[:])
        if split_store:
            # Two parallel half-row stores shorten the tail-exposed transfer.
            # Both go on hardware-DGE rings (SP + Act): the gpsimd queue is
            # SWDGE, whose slower completion gated the kernel's teardown.
            nc.sync.dma_start(ot[h, 0:33, qs], o_sb[0:33, :])
            nc.scalar.dma_start(ot[h, 33:65, qs], o_sb[33:65, :])
        else:
            nc.sync.dma_start(ot[h, :, qs], o_sb[:])

    def emit_warmup(n_mm=42):
        """HAM pre-warm: the PE clock-gate defaults to half rate and only
        lifts after ~3.4us of SUSTAINED activity; any idle window resets it
        (in-order PE dispatch makes mid-stream gap-filling impossible — a
        fill emitted after a gated matmul inherits its gate; measured).
        Fill the initial DMA-wait gap with tiny zero x zero matmuls into a
        scratch PSUM slot."""
        wz = inp.tile([128, 64], mm_dt, tag="warm", name="warm0")
        nc.gpsimd.memset(wz[:], 0)
        scr = opsum.tile([65, 512], mybir.dt.float32, tag="po", name="warmps")
        for _ in range(n_mm):
            nc.tensor.matmul(
                scr[0:64, 0:64], lhsT=wz[0:64, :], rhs=wz[0:64, :],
                start=True, stop=True,
            )

    def emit_s_group(h, qh, gi, kt_at, qt_at, kc0, glen, eng):
        """S^T matmuls + exp for one k-chunk group of one (head, q-half).
        Even kc use array rows 0-63, odd kc rows 64-127 (tile_position
        auto-derived from base partitions), so each aligned even/odd pair
        of matmuls runs concurrently on the PE. `eng` picks the exp engine:
        'sc' = ScalarE exact exp, 'dve' = VectorE Schraudolph fp16."""
        ps = spsum.tile(
            [128, glen * 512], mybir.dt.float32, tag="ps", name=f"ps{h}_{qh}_{gi}"
        )
        if glen == 2 and kc0 % 2 == 0:
            # Pair-aligned group: ONE full-array LDWEIGHTS loads both chunks'
            # weights (cost ~= columns, so one 128x128 load == one 64x128
            # load), then two non-self-loading matmuls stream the two
            # partition halves concurrently. Cuts the PE instruction count
            # per pair 4 -> 3 and halves the weight-load time (the S pair is
            # instruction-rate-bound, not stream-bound).
            jp = kc0 // 2
            nc.tensor.ldweights(kt_at(slice(0, 128), jp))
            for c in range(2):
                rows = slice(c * 64, c * 64 + 64)
                mm = nc.tensor.matmul(
                    ps[:, c * 512 : (c + 1) * 512],
                    lhsT=kt_at(rows, jp),
                    rhs=qt_at(rows, qh),
                    start=True,
                    stop=True,
                )
                mm.ldweights = False
        else:
            for c in range(glen):
                kc = kc0 + c
                jp, half = divmod(kc, 2)
                rows = slice(half * 64, half * 64 + 64)
                nc.tensor.matmul(
                    ps[:, c * 512 : (c + 1) * 512],
                    lhsT=kt_at(rows, jp),
                    rhs=qt_at(rows, qh),
                    start=True,
                    stop=True,
                )
        if eng == "sc":
            pt = pts.tile([128, glen * 512], mm_dt, tag="pt", name=f"pt{h}_{qh}_{gi}")
            nc.scalar.activation(
                pt[:], ps[:], mybir.ActivationFunctionType.Exp, scale=scale
            )
        else:
            pt = pti.tile(
                [128, glen * 512], mybir.dt.int16, tag="pti", name=f"pi{h}_{qh}_{gi}"
            )
            nc.vector.tensor_scalar(
                pt[:], ps[:], a_dve, b_dve,
                mybir.AluOpType.mult, mybir.AluOpType.add,
            )
        return [(kc0 + c, pt, c * 512) for c in range(glen)]

    # Software pipeline over (head, q-half) units with a skew of 2. Unit
    # i-2's O matmuls are emitted in two 4-chunk batches woven between unit
    # i's S groups, so the PE never sits on a full-PSUM wait with O work
    # stuck behind it in queue order, and consumers get S tiles early.
    # The very first unit splits its first pair-group so the first exp fires
    # after a single matmul — ScalarE's busy span starts earlier.
    # The last iteration drains all remaining O work as early as its exp
    # dependencies allow, leaving only 2 matmuls + copy + store after the
    # final exp.
    first_groups = [(0, 1, "sc"), (1, 1, "sc"), (2, 2, "dve"), (4, 2, "sc"),
                    (6, 2, "dve")]
    units = [(h, qh) for h in range(HPC) for qh in range(QH)]
    tiles = {}
    inflight = []

    def o_tile(h, qh):
        return opsum.tile([65, 512], mybir.dt.float32, tag="po", name=f"po{h}_{qh}")

    for i, (h, qh) in enumerate(units):
        last_it = i == len(units) - 1
        if qh == 0:
            if h == 0:
                emit_warmup()
            tiles[h] = emit_loads(h)
        kt_at, qt_at, vp_t = tiles[h]
        if i == 0:
            groups = first_groups
        else:
            pat = _PAT_A if i % 2 == 0 else _PAT_B
            groups = [(g * 2, 2, pat[g]) for g in range(4)]
        pend = inflight.pop(0) if len(inflight) > 1 else None
        if pend is not None:
            ph, pqh, pvp, pch = pend
            po_p = o_tile(ph, pqh)
        chunks = []
        done_o = 0
        for gi, (kc0, glen, eng) in enumerate(groups):
            if last_it and gi == len(groups) - 1:
                # Drain: all remaining O work is emitted as early as its exp
                # dependencies allow, leaving only 2 matmuls + copy + store
                # after the final exp.
                if pend is not None:
                    emit_o_mms(ph, pqh, pvp, pch[done_o:8], po_p)
                    emit_o_out(ph, pqh, po_p)
                h2, qh2, vp2, ch2 = inflight.pop(0)
                po2 = o_tile(h2, qh2)
                emit_o_mms(h2, qh2, vp2, ch2, po2)
                emit_o_out(h2, qh2, po2, split_store=True)
                po3 = o_tile(h, qh)
                emit_o_mms(h, qh, vp_t, chunks[0:6], po3)
                chunks += emit_s_group(h, qh, gi, kt_at, qt_at, kc0, glen, eng)
                emit_o_mms(h, qh, vp_t, chunks[6:8], po3)
                emit_o_out(h, qh, po3, copy_eng="split", split_store=True)
                break
            chunks += emit_s_group(h, qh, gi, kt_at, qt_at, kc0, glen, eng)
            if pend is not None and gi in (1, 2):
                # Two 4-chunk O batches woven between S groups: keeps the PE
                # queue free of full-PSUM waits without paying too many
                # S<->O accumulation-context switches (2-chunk batches after
                # every group measured slower). The copy-out goes right after
                # the last batch — before the final S group — so the opsum
                # slot frees a pair-slot earlier and the DVE copy overlaps g3.
                emit_o_mms(ph, pqh, pvp, pch[done_o : done_o + 4], po_p)
                done_o += 4
                if gi == 2:
                    emit_o_out(ph, pqh, po_p)
                    pend = None
        if not last_it:
            inflight.append((h, qh, vp_t, chunks))


def _build(mm_dt, scale):
    nc = bacc.Bacc(
        "TRN2",
        target_bir_lowering=False,
        debug=False,
        enable_asserts=False,
        num_devices=NCORES,
    )
    qt_d = nc.dram_tensor("qt", [HPC, 128, N], mm_dt, kind="ExternalInput")
    kt_d = nc.dram_tensor("kt", [HPC, 128, JP * 128], mm_dt, kind="ExternalInput")
    vp_d = nc.dram_tensor("vp", [HPC, 128, KC * 65], mm_dt, kind="ExternalInput")
    qk0a_d = nc.dram_tensor("qk0a", [1, 128, 768], mm_dt, kind="ExternalInput")
    qk0b_d = nc.dram_tensor("qk0b", [1, 128, 768], mm_dt, kind="ExternalInput")
    ot_d = nc.dram_tensor("ot", [HPC, 65, N], mybir.dt.float32, kind="ExternalOutput")
    with tile.TileContext(nc) as tc:
        with ExitStack() as ctx:
            _emit(ctx, tc, qt_d.ap(), kt_d.ap(), vp_d.ap(), qk0a_d.ap(), qk0b_d.ap(), ot_d.ap(), mm_dt, scale)
    nc.compile()
    return nc


def _get_nc(mm_dt, scale):
    key = (mm_dt, scale)
    if key not in _NC_CACHE:
        _NC_CACHE[key] = _build(mm_dt, scale)
    return _NC_CACHE[key]


def kernel(Q, K, V, qkv=None, **_unused):
    global LAST_RESULTS
    Q = np.asarray(Q, dtype=np.float32)
    K = np.asarray(K, dtype=np.float32)
    V = np.asarray(V, dtype=np.float32)

    # Host-side layout prep (not part of HW exec time).
    Qt = Q.transpose(0, 1, 3, 2)                       # [B, H, D, N]
    QtD = np.concatenate([Qt, Qt], axis=2)             # [B, H, 128, N]
    Kt = K.transpose(0, 1, 3, 2)                       # [B, H, D, N]
    KtP = (
        Kt.reshape(B, H, D, JP, 2, 128)
        .transpose(0, 1, 4, 2, 3, 5)
        .reshape(B, H, 128, JP * 128)
    )
    Vp = np.ones((B, H, 128, KC * 65), dtype=np.float32)
    Vp.reshape(B, H, 128, KC, 65)[..., :64] = V.reshape(B, H, KC, 128, D).transpose(
        0, 1, 3, 2, 4
    )

    QtD = QtD.astype(np.float16)
    KtP = KtP.astype(np.float16)
    Vp = Vp.astype(np.float16)

    trace = bool(int(os.environ.get("ATT_TRACE", "0")))
    if trace:
        _install_ntff_hook()
    scale = 1.0 / float(np.sqrt(np.float64(int(qkv)))) if qkv is not None else (
        1.0 / float(np.sqrt(np.float64(D)))
    )
    nc = _get_nc(_MM_DT, scale)
    in_maps = [
        {
            "qt": np.ascontiguousarray(QtD[c]),
            "kt": np.ascontiguousarray(KtP[c]),
            "vp": np.ascontiguousarray(Vp[c]),
            "qk0a": np.ascontiguousarray(
                np.concatenate(
                    [KtP[c, 0, :, 0:256], QtD[c, 0, :, 0:512]], axis=-1
                )[None]
            ),
            "qk0b": np.ascontiguousarray(
                np.concatenate(
                    [KtP[c, 0, :, 256:512], QtD[c, 0, :, 512:1024]], axis=-1
                )[None]
            ),
        }
        for c in range(NCORES)
    ]
    res = run_bass_kernel_spmd(
        nc,
        in_maps,
        core_ids=list(range(NCORES)),
        trace=trace,
    )
    LAST_RESULTS = res

    out = np.empty((B, H, N, D), dtype=np.float32)
    for c in range(NCORES):
        ot = res.results[c]["ot"]                      # [HPC, 65, N]
        denom = ot[:, 64:65, :]                        # [HPC, 1, N]
        out[c] = (ot[:, :64, :] / denom).transpose(0, 2, 1)
    return out


# revision 20
# speedup vs baseline: 1.0257x; 1.0257x over previous
"""Multi-head attention kernel for 8 Trainium2 NeuronCores.

Problem: O = softmax(Q @ K^T / sqrt(D)) @ V with B=8, H=12, N=1024, D=64, fp32.

Sharding: batch dim across the 8 cores (12 heads per core) — attention is
embarrassingly parallel over (b, h).

Device-side layout (host prep is free — only HW exec time counts):
  - Q, K are pre-transposed on host to [D, N] so the d-contraction of
    S = Q @ K^T has d on SBUF partitions for both operands.
  - S is computed *transposed* (S^T[k, q], k on partitions) so that the
    second matmul O^T = (V | 1)^T @ P^T needs no on-chip transposes at all.
  - Q^T is duplicated onto both partition halves and K^T chunks are packed
    in (even, odd) pairs on partition halves 0-63 / 64-127: the two K=64
    matmuls of a pair auto-derive tile_position (0,0)/(64,0) and run
    concurrently in the two row-halves of the PE array.
  - exp() is split across TWO engines so ScalarE stops being the pacing
    engine (exp is 98304 elem/lane; ScalarE alone = 81.9us @ 1.2GHz):
      * ScalarE: exact exp out of PSUM with the 1/sqrt(D) scale folded in.
      * VectorE (DVE): Schraudolph fp16 exp — one tensor_scalar
        (i16 = rint(s*A + B), A = 1024*log2(e)*scale, B = 15360 - 59) whose
        int16 bit pattern IS the fp16 approximation of exp(s*scale); the
        O matmul reads the tile bitcast to fp16. The -59 offset centers the
        piecewise-linear ripple so mixed exact/approx softmax rows carry no
        systematic bias (measured end-to-end rel err ~1.1e-2 at the default
        3/8 DVE share, vs the 2e-2 gate).
  - S^T chunk groups are pair-aligned (2 k-chunks = one concurrent PE pair
    per group, [128, 1024] fp32 = 2 PSUM banks, ring of 3) so every S
    matmul pair runs 2x on the PE array.
  - V gets a ones-column appended (65th weight column), so the P^T @ (V|1)
    matmul emits the softmax denominator as output row 64 for free.
  - The PE clock-gate (HAM) defaults to half rate; dummy warmup matmuls
    fill the initial input-DMA wait so the ramp starts as early as possible.
  - Normalization (divide by row 64) + final transpose happen on host.
"""

import os
from contextlib import ExitStack

import numpy as np

import concourse.tile as tile
from concourse import bacc, mybir
from concourse.bass_utils import run_bass_kernel_spmd

B, H, N, D = 8, 12, 1024, 64
NCORES = 8
HPC = B // NCORES * H  # heads per core = 12 (one full batch row per core)
KC = N // 128          # 8 key chunks of 128
JP = KC // 2           # 4 chunk pairs
QH = N // 512          # 2 query halves of 512

# Matmul dtype: fp16 streams at 1 cycle/row on the PE and halves DMA/SBUF.
_MM_DT = mybir.dt.float16

# Schraudolph bias offset (in 1/1024 fp16-exponent units): centers the
# piecewise-linear exp ripple so mixed exact/approx rows are unbiased.
_SCHR_OFF = float(os.environ.get("ATT_SCHR_OFF", "59"))

# Per-unit engine patterns for the 4 pair-groups (chunks 01|23|45|67).
# Alternating 1-dve / 2-dve units => 3/8 of exp on DVE. The global group
# sequence alternates engines (…sc,dve,sc,sc,dve,sc,dve,sc…) so the shared
# 3-deep PSUM ring is never drained by one engine's backlog.
_PAT_A = ("sc", "dve", "sc", "sc")
_PAT_B = ("dve", "sc", "dve", "sc")

LAST_RESULTS = None
_NC_CACHE = {}


def _install_ntff_hook():
    """Register the axon NTFF profile hook (the agent image's antenv lacks
    axon_hooks, so trn_boot degrades silently). Best-effort: tracing only."""
    import sys
    import types

    if "antenv.axon_hooks" in sys.modules:
        return
    try:
        import contextlib
        import ctypes

        so_path = "/opt/axon/libaxon_pjrt.so"
        lib = ctypes.CDLL(so_path)
        if not hasattr(lib, "axon_start_nrt_profile"):
            return
        lib.axon_start_nrt_profile.argtypes = [
            ctypes.POINTER(ctypes.c_int64),
            ctypes.c_size_t,
        ]
        lib.axon_start_nrt_profile.restype = ctypes.c_int64
        lib.axon_stop_nrt_profile.argtypes = [ctypes.c_char_p]
        lib.axon_stop_nrt_profile.restype = ctypes.c_int64

        @contextlib.contextmanager
        def _hook(output_dir, device_ids):
            import jax

            jax.devices()
            if device_ids:
                ids = (ctypes.c_int64 * len(device_ids))(*device_ids)
                rc = lib.axon_start_nrt_profile(ids, len(device_ids))
            else:
                rc = lib.axon_start_nrt_profile(None, 0)
            if rc != 0:
                raise RuntimeError(f"axon_start_nrt_profile rc={rc}")
            try:
                yield
            finally:
                n = lib.axon_stop_nrt_profile(str(output_dir).encode())
                print(f"ntff profile: {n} file(s) written to {output_dir}")

        mod = types.ModuleType("antenv.axon_hooks")
        mod.get_axon_ntff_profile_hook = lambda: _hook
        mod.set_axon_ntff_profile_hook = lambda h: None
        sys.modules["antenv.axon_hooks"] = mod
    except Exception:
        pass


def _emit(ctx, tc, qt, kt, vp, qk0a, qk0b, ot, mm_dt, scale):
    nc = tc.nc
    inp = ctx.enter_context(tc.tile_pool(name="inp", bufs=3))
    pts = ctx.enter_context(tc.tile_pool(name="pts", bufs=9))
    pti = ctx.enter_context(tc.tile_pool(name="pti", bufs=7))
    outp = ctx.enter_context(tc.tile_pool(name="outp", bufs=4))
    spsum = ctx.enter_context(tc.tile_pool(name="spsum", bufs=3, space="PSUM"))
    opsum = ctx.enter_context(tc.tile_pool(name="opsum", bufs=2, space="PSUM"))

    a_dve = float(1024.0 * np.log2(np.e) * scale)
    b_dve = float(15360.0 - _SCHR_OFF)

    def emit_loads(h):
        """Input DMA loads for head h. kt before qt (LDWEIGHTS consumes kt
        first); vp on the gpsimd queue — it is only needed by the O phase and
        must not delay the S loads."""
        if h == 0:
            # Head 0: K^T/Q^T arrive as host-packed DMAs split across
            # queues so the first S group's operands land with parallel
            # transfers — the head is DMA-latency bound. (Only SP issues
            # here: the Act ring is blocked by ACT_TABLE_LOAD until ~9us
            # and DVE cannot issue DMAs.)
            qk_a = inp.tile([128, 768], mm_dt, tag="qk0a", name="qk0a")
            nc.sync.dma_start(qk_a[:], qk0a[0])
            qk_b = inp.tile([128, 768], mm_dt, tag="qk0b", name="qk0b")
            # kt pairs 2-3 ride the fast SP ring right behind qk0a (unit 0's
            # g2/g3 are gated on them); qt q-half 1 is only needed by unit 1
            # and can take the slower SWDGE ring.
            nc.sync.dma_start(qk_b[:, 0:256], qk0b[0, :, 0:256])
            nc.gpsimd.dma_start(qk_b[:, 256:768], qk0b[0, :, 256:768])

            def kt_at(rows, jp):
                if jp < 2:
                    return qk_a[rows, jp * 128 : (jp + 1) * 128]
                return qk_b[rows, (jp - 2) * 128 : (jp - 1) * 128]

            def qt_at(rows, qh):
                src_t = qk_a if qh == 0 else qk_b
                return src_t[rows, 256:768]
        else:
            kt_t = inp.tile([128, JP * 128], mm_dt, tag="kt", name=f"kt{h}")
            nc.sync.dma_start(kt_t[:], kt[h])
            qt_t = inp.tile([128, N], mm_dt, tag="qt", name=f"qt{h}")
            nc.sync.dma_start(qt_t[:, 0:512], qt[h, :, 0:512])
            nc.sync.dma_start(qt_t[:, 512:1024], qt[h, :, 512:1024])

            def kt_at(rows, jp):
                return kt_t[rows, jp * 128 : (jp + 1) * 128]

            def qt_at(rows, qh):
                if qh is None:
                    return qt_t[rows, 0:1024]
                return qt_t[rows, qh * 512 : (qh + 1) * 512]

        vp_t = inp.tile([128, KC * 65], mm_dt, tag="vp", name=f"vp{h}")
        nc.gpsimd.dma_start(vp_t[:], vp[h])
        return kt_at, qt_at, vp_t

    def emit_o_mms(h, qh, vp_t, chunks, po):
        """O^T accumulation matmuls for a subset of k-chunks of one unit.
        O^T[m, q] = sum_k (V|1)[k, m] * P^T[k, q]; row 64 is the softmax
        denominator. `chunks` is [(kc, pt, off)] — each chunk's exp lives
        at column offset `off` of (possibly shared, cross-unit) pt tiles."""
        for kc, pt, off in chunks:
            rhs = pt[:, off : off + 512]
            if rhs.dtype == mybir.dt.int16:
                rhs = rhs.bitcast(mybir.dt.float16)
            nc.tensor.matmul(
                po[:],
                lhsT=vp_t[:, kc * 65 : (kc + 1) * 65],
                rhs=rhs,
                start=(kc == 0),
                stop=(kc == KC - 1),
            )

    def emit_o_out(h, qh, po, copy_eng="v", split_store=False):
        """PSUM->SBUF copy-out + store for one finished O^T accumulation."""
        qs = slice(qh * 512, (qh + 1) * 512)
        o_sb = outp.tile([65, 512], mybir.dt.float32, tag="osb", name=f"ou{h}_{qh}")
        if copy_eng == "v":
            nc.vector.tensor_copy(o_sb[:], po[:])
        elif copy_eng == "split":
            nc.vector.tensor_copy(o_sb[:, 0:256], po[:, 0:256])
            nc.scalar.copy(o_sb[:, 256:512], po[:, 256:512])
        else:
            nc.scalar.copy(o_sb[:], po[:])
        if split_store:
            # Two parallel half-row stores shorten the tail-exposed transfer.
            # Both go on hardware-DGE rings (SP + Act): the gpsimd queue is
            # SWDGE, whose slower completion gated the kernel's teardown.
            nc.sync.dma_start(ot[h, 0:33, qs], o_sb[0:33, :])
            nc.scalar.dma_start(ot[h, 33:65, qs], o_sb[33:65, :])
        else:
            nc.sync.dma_start(ot[h, :, qs], o_sb[:])

    def emit_warmup(n_mm=42):
        """HAM pre-warm: the PE clock-gate defaults to half rate and only
        lifts after ~3.4us of SUSTAINED activity; any idle window resets it
        (in-order PE dispatch makes mid-stream gap-filling impossible — a
        fill emitted after a gated matmul inherits its gate; measured).
        Fill the initial DMA-wait gap with tiny zero x zero matmuls into a
        scratch PSUM slot."""
        wz = inp.tile([128, 64], mm_dt, tag="warm", name="warm0")
        nc.gpsimd.memset(wz[:], 0)
        scr = opsum.tile([65, 512], mybir.dt.float32, tag="po", name="warmps")
        for _ in range(n_mm):
            nc.tensor.matmul(
                scr[0:64, 0:64], lhsT=wz[0:64, :], rhs=wz[0:64, :],
                start=True, stop=True,
            )

    def emit_s_group(h, qh, gi, kt_at, qt_at, kc0, glen, eng):
        """S^T matmuls + exp for one k-chunk group of one (head, q-half).
        Even kc use array rows 0-63, odd kc rows 64-127 (tile_position
        auto-derived from base partitions), so each aligned even/odd pair
        of matmuls runs concurrently on the PE. `eng` picks the exp engine:
        'sc' = ScalarE exact exp, 'dve' = VectorE Schraudolph fp16."""
        ps = spsum.tile(
            [128, glen * 512], mybir.dt.float32, tag="ps", name=f"ps{h}_{qh}_{gi}"
        )
        # NOTE: a shared full-array LDWEIGHTS + two non-self-loading matmuls
        # (mm.ldweights = False) is numerically correct but measured ~9us
        # SLOWER: the [128,128] load conflicts with every quadrant, so it
        # cannot be pulled into the background weight buffer while the
        # previous pair streams. Per-matmul half-array loads overlap better.
        for c in range(glen):
            kc = kc0 + c
            jp, half = divmod(kc, 2)
            rows = slice(half * 64, half * 64 + 64)
            nc.tensor.matmul(
                ps[:, c * 512 : (c + 1) * 512],
                lhsT=kt_at(rows, jp),
                rhs=qt_at(rows, qh),
                start=True,
                stop=True,
            )
        if eng == "sc":
            pt = pts.tile([128, glen * 512], mm_dt, tag="pt", name=f"pt{h}_{qh}_{gi}")
            nc.scalar.activation(
                pt[:], ps[:], mybir.ActivationFunctionType.Exp, scale=scale
            )
        else:
            pt = pti.tile(
                [128, glen * 512], mybir.dt.int16, tag="pti", name=f"pi{h}_{qh}_{gi}"
            )
            nc.vector.tensor_scalar(
                pt[:], ps[:], a_dve, b_dve,
                mybir.AluOpType.mult, mybir.AluOpType.add,
            )
        return [(kc0 + c, pt, c * 512) for c in range(glen)]

    def _exp_full(h, kc, ps, eng):
        if eng == "sc":
            pt = pts.tile([128, 1024], mm_dt, tag="pt", name=f"pf{h}_{kc}")
            nc.scalar.activation(
                pt[:], ps[:], mybir.ActivationFunctionType.Exp, scale=scale
            )
        else:
            pt = pti.tile([128, 1024], mybir.dt.int16, tag="pti", name=f"pf{h}_{kc}")
            nc.vector.tensor_scalar(
                pt[:], ps[:], a_dve, b_dve,
                mybir.AluOpType.mult, mybir.AluOpType.add,
            )
        return pt

    def emit_s_pair(h, kc0, kt_at, qt_at, eng_e, eng_o):
        """S^T for one even/odd chunk pair across the FULL 1024 q, with one
        weight load per chunk: each chunk's second (q-half 1) matmul reuses
        the weights its first matmul loaded (ldweights=False — partial
        row-half loads don't disturb the other half's weights, proven by the
        baseline pairing). Emission order e1,o1,e2,o2 keeps both PE
        row-halves streaming: ~431ns per pair vs ~660ns as two 512-wide
        pairs (LDWEIGHTS fully hidden, 2 loads instead of 4)."""
        jp = kc0 // 2
        rows_e = slice(0, 64)
        rows_o = slice(64, 128)
        ps_e = spsum.tile([128, 1024], mybir.dt.float32, tag="ps", name=f"pe{h}_{kc0}")
        ps_o = spsum.tile([128, 1024], mybir.dt.float32, tag="ps", name=f"po{h}_{kc0}")
        nc.tensor.matmul(ps_e[:, 0:512], lhsT=kt_at(rows_e, jp),
                         rhs=qt_at(rows_e, 0), start=True, stop=True)
        nc.tensor.matmul(ps_o[:, 0:512], lhsT=kt_at(rows_o, jp),
                         rhs=qt_at(rows_o, 0), start=True, stop=True)
        m2e = nc.tensor.matmul(ps_e[:, 512:1024], lhsT=kt_at(rows_e, jp),
                               rhs=qt_at(rows_e, 1), start=True, stop=True)
        m2e.ldweights = False
        m2o = nc.tensor.matmul(ps_o[:, 512:1024], lhsT=kt_at(rows_o, jp),
                               rhs=qt_at(rows_o, 1), start=True, stop=True)
        m2o.ldweights = False
        return _exp_full(h, kc0, ps_e, eng_e), _exp_full(h, kc0 + 1, ps_o, eng_o)

    # Software pipeline over (head, q-half) units with a skew of 2. Unit
    # i-2's O matmuls are emitted in two 4-chunk batches woven between unit
    # i's S groups, so the PE never sits on a full-PSUM wait with O work
    # stuck behind it in queue order, and consumers get S tiles early.
    # The very first unit splits its first pair-group so the first exp fires
    # after a single matmul — ScalarE's busy span starts earlier.
    # The last iteration drains all remaining O work as early as its exp
    # dependencies allow, leaving only 2 matmuls + copy + store after the
    # final exp.
    first_groups = [(0, 1, "sc"), (1, 1, "sc"), (2, 2, "dve"), (4, 2, "sc"),
                    (6, 2, "dve")]
    # Heads 1-11: one S matmul per k-chunk streaming the FULL 1024 q (adjacent
    # even/odd chunks still pair on the PE via partition-half tile_position);
    # 3/8 of chunks take the DVE Schraudolph path.
    _PAT8 = ("sc", "dve", "sc", "sc", "dve", "sc", "dve", "sc")
    tiles = {}
    inflight = []

    def o_tile(h, qh):
        return opsum.tile([65, 512], mybir.dt.float32, tag="po", name=f"po{h}_{qh}")

    def drain_unit(u, copy_eng="v", split_store=False):
        uh, uqh, uvp, uch = u
        po = o_tile(uh, uqh)
        emit_o_mms(uh, uqh, uvp, uch, po)
        emit_o_out(uh, uqh, po, copy_eng=copy_eng, split_store=split_store)

    for h in range(HPC):
        if h == 0:
            emit_warmup()
        tiles[h] = emit_loads(h)
        kt_at, qt_at, vp_t = tiles[h]
        if h == 0:
            # Head 0 keeps 512-wide streams: its q-halves arrive as separate
            # parallel transfers, and the split first groups start ScalarE as
            # early as possible.
            for qh in range(QH):
                groups = first_groups if qh == 0 else [
                    (g * 2, 2, _PAT_B[g]) for g in range(4)
                ]
                chunks = []
                for gi, (kc0, glen, eng) in enumerate(groups):
                    chunks += emit_s_group(h, qh, gi, kt_at, qt_at, kc0, glen, eng)
                inflight.append((h, qh, vp_t, chunks))
            continue
        last_h = h == HPC - 1
        pA = inflight.pop(0)
        pB = inflight.pop(0)
        hA, qhA, vpA, chA = pA
        hB, qhB, vpB, chB = pB
        new_pt = {}
        new_pt[0], new_pt[1] = emit_s_pair(h, 0, kt_at, qt_at, _PAT8[0], _PAT8[1])
        poA = o_tile(hA, qhA)
        emit_o_mms(hA, qhA, vpA, chA[0:4], poA)
        new_pt[2], new_pt[3] = emit_s_pair(h, 2, kt_at, qt_at, _PAT8[2], _PAT8[3])
        emit_o_mms(hA, qhA, vpA, chA[4:8], poA)
        emit_o_out(hA, qhA, poA)
        new_pt[4], new_pt[5] = emit_s_pair(h, 4, kt_at, qt_at, _PAT8[4], _PAT8[5])
        poB = o_tile(hB, qhB)
        emit_o_mms(hB, qhB, vpB, chB[0:4], poB)
        new_pt[6], new_pt[7] = emit_s_pair(h, 6, kt_at, qt_at, _PAT8[6], _PAT8[7])
        emit_o_mms(hB, qhB, vpB, chB[4:8], poB)
        emit_o_out(hB, qhB, poB)
        inflight.append((h, 0, vp_t, [(kc, new_pt[kc], 0) for kc in range(KC)]))
        inflight.append((h, 1, vp_t, [(kc, new_pt[kc], 512) for kc in range(KC)]))
        if last_h:
            # Drain the final head's own units; the very last copy/store is
            # split across both engines/rings to shorten the tail.
            drain_unit(inflight.pop(0), split_store=True)
            drain_unit(inflight.pop(0), copy_eng="split", split_store=True)


def _build(mm_dt, scale):
    nc = bacc.Bacc(
        "TRN2",
        target_bir_lowering=False,
        debug=False,
        enable_asserts=False,
        num_devices=NCORES,
    )
    qt_d = nc.dram_tensor("qt", [HPC, 128, N], mm_dt, kind="ExternalInput")
    kt_d = nc.dram_tensor("kt", [HPC, 128, JP * 128], mm_dt, kind="ExternalInput")
    vp_d = nc.dram_tensor("vp", [HPC, 128, KC * 65], mm_dt, kind="ExternalInput")
    qk0a_d = nc.dram_tensor("qk0a", [1, 128, 768], mm_dt, kind="ExternalInput")
    qk0b_d = nc.dram_tensor("qk0b", [1, 128, 768], mm_dt, kind="ExternalInput")
    ot_d = nc.dram_tensor("ot", [HPC, 65, N], mybir.dt.float32, kind="ExternalOutput")
    with tile.TileContext(nc) as tc:
        with ExitStack() as ctx:
            _emit(ctx, tc, qt_d.ap(), kt_d.ap(), vp_d.ap(), qk0a_d.ap(), qk0b_d.ap(), ot_d.ap(), mm_dt, scale)
    nc.compile()
    return nc


def _get_nc(mm_dt, scale):
    key = (mm_dt, scale)
    if key not in _NC_CACHE:
        _NC_CACHE[key] = _build(mm_dt, scale)
    return _NC_CACHE[key]


def kernel(Q, K, V, qkv=None, **_unused):
    global LAST_RESULTS
    Q = np.asarray(Q, dtype=np.float32)
    K = np.asarray(K, dtype=np.float32)
    V = np.asarray(V, dtype=np.float32)

    # Host-side layout prep (not part of HW exec time).
    Qt = Q.transpose(0, 1, 3, 2)                       # [B, H, D, N]
    QtD = np.concatenate([Qt, Qt], axis=2)             # [B, H, 128, N]
    Kt = K.transpose(0, 1, 3, 2)                       # [B, H, D, N]
    KtP = (
        Kt.reshape(B, H, D, JP, 2, 128)
        .transpose(0, 1, 4, 2, 3, 5)
        .reshape(B, H, 128, JP * 128)
    )
    Vp = np.ones((B, H, 128, KC * 65), dtype=np.float32)
    Vp.reshape(B, H, 128, KC, 65)[..., :64] = V.reshape(B, H, KC, 128, D).transpose(
        0, 1, 3, 2, 4
    )

    QtD = QtD.astype(np.float16)
    KtP = KtP.astype(np.float16)
    Vp = Vp.astype(np.float16)

    trace = bool(int(os.environ.get("ATT_TRACE", "0")))
    if trace:
        _install_ntff_hook()
    scale = 1.0 / float(np.sqrt(np.float64(int(qkv)))) if qkv is not None else (
        1.0 / float(np.sqrt(np.float64(D)))
    )
    nc = _get_nc(_MM_DT, scale)
    in_maps = [
        {
            "qt": np.ascontiguousarray(QtD[c]),
            "kt": np.ascontiguousarray(KtP[c]),
            "vp": np.ascontiguousarray(Vp[c]),
            "qk0a": np.ascontiguousarray(
                np.concatenate(
                    [KtP[c, 0, :, 0:256], QtD[c, 0, :, 0:512]], axis=-1
                )[None]
            ),
            "qk0b": np.ascontiguousarray(
                np.concatenate(
                    [KtP[c, 0, :, 256:512], QtD[c, 0, :, 512:1024]], axis=-1
                )[None]
            ),
        }
        for c in range(NCORES)
    ]
    res = run_bass_kernel_spmd(
        nc,
        in_maps,
        core_ids=list(range(NCORES)),
        trace=trace,
    )
    LAST_RESULTS = res

    out = np.empty((B, H, N, D), dtype=np.float32)
    for c in range(NCORES):
        ot = res.results[c]["ot"]                      # [HPC, 65, N]
        denom = ot[:, 64:65, :]                        # [HPC, 1, N]
        out[c] = (ot[:, :64, :] / denom).transpose(0, 2, 1)
    return out


# revision 21
# speedup vs baseline: 1.0831x; 1.0560x over previous
"""Multi-head attention kernel for 8 Trainium2 NeuronCores.

Problem: O = softmax(Q @ K^T / sqrt(D)) @ V with B=8, H=12, N=1024, D=64, fp32.

Sharding: batch dim across the 8 cores (12 heads per core) — attention is
embarrassingly parallel over (b, h).

Device-side layout (host prep is free — only HW exec time counts):
  - Q, K are pre-transposed on host to [D, N] so the d-contraction of
    S = Q @ K^T has d on SBUF partitions for both operands.
  - S is computed *transposed* (S^T[k, q], k on partitions) so that the
    second matmul O^T = (V | 1)^T @ P^T needs no on-chip transposes at all.
  - Q^T is duplicated onto both partition halves and K^T chunks are packed
    in (even, odd) pairs on partition halves 0-63 / 64-127: the two K=64
    matmuls of a pair auto-derive tile_position (0,0)/(64,0) and run
    concurrently in the two row-halves of the PE array.
  - exp() is split across TWO engines so ScalarE stops being the pacing
    engine (exp is 98304 elem/lane; ScalarE alone = 81.9us @ 1.2GHz):
      * ScalarE: exact exp out of PSUM with the 1/sqrt(D) scale folded in.
      * VectorE (DVE): Schraudolph fp16 exp — one tensor_scalar
        (i16 = rint(s*A + B), A = 1024*log2(e)*scale, B = 15360 - 59) whose
        int16 bit pattern IS the fp16 approximation of exp(s*scale); the
        O matmul reads the tile bitcast to fp16. The -59 offset centers the
        piecewise-linear ripple so mixed exact/approx softmax rows carry no
        systematic bias (measured end-to-end rel err ~1.1e-2 at the default
        3/8 DVE share, vs the 2e-2 gate).
  - S^T chunk groups are pair-aligned (2 k-chunks = one concurrent PE pair
    per group, [128, 1024] fp32 = 2 PSUM banks, ring of 3) so every S
    matmul pair runs 2x on the PE array.
  - V gets a ones-column appended (65th weight column), so the P^T @ (V|1)
    matmul emits the softmax denominator as output row 64 for free.
  - The PE clock-gate (HAM) defaults to half rate; dummy warmup matmuls
    fill the initial input-DMA wait so the ramp starts as early as possible.
  - Normalization (divide by row 64) + final transpose happen on host.
"""

import os
from contextlib import ExitStack

import numpy as np

import concourse.tile as tile
from concourse import bacc, mybir
from concourse.bass_utils import run_bass_kernel_spmd

B, H, N, D = 8, 12, 1024, 64
NCORES = 8
HPC = B // NCORES * H  # heads per core = 12 (one full batch row per core)
KC = N // 128          # 8 key chunks of 128
JP = KC // 2           # 4 chunk pairs
QH = N // 512          # 2 query halves of 512

# Matmul dtype: fp16 streams at 1 cycle/row on the PE and halves DMA/SBUF.
_MM_DT = mybir.dt.float16

# Schraudolph bias offset (in 1/1024 fp16-exponent units): centers the
# piecewise-linear exp ripple so mixed exact/approx rows are unbiased.
_SCHR_OFF = float(os.environ.get("ATT_SCHR_OFF", "59"))

# Per-unit engine patterns for the 4 pair-groups (chunks 01|23|45|67).
# Alternating 1-dve / 2-dve units => 3/8 of exp on DVE. The global group
# sequence alternates engines (…sc,dve,sc,sc,dve,sc,dve,sc…) so the shared
# 3-deep PSUM ring is never drained by one engine's backlog.
_PAT_A = ("sc", "dve", "sc", "sc")
_PAT_B = ("dve", "sc", "dve", "sc")

LAST_RESULTS = None
_NC_CACHE = {}


def _install_ntff_hook():
    """Register the axon NTFF profile hook (the agent image's antenv lacks
    axon_hooks, so trn_boot degrades silently). Best-effort: tracing only."""
    import sys
    import types

    if "antenv.axon_hooks" in sys.modules:
        return
    try:
        import contextlib
        import ctypes

        so_path = "/opt/axon/libaxon_pjrt.so"
        lib = ctypes.CDLL(so_path)
        if not hasattr(lib, "axon_start_nrt_profile"):
            return
        lib.axon_start_nrt_profile.argtypes = [
            ctypes.POINTER(ctypes.c_int64),
            ctypes.c_size_t,
        ]
        lib.axon_start_nrt_profile.restype = ctypes.c_int64
        lib.axon_stop_nrt_profile.argtypes = [ctypes.c_char_p]
        lib.axon_stop_nrt_profile.restype = ctypes.c_int64

        @contextlib.contextmanager
        def _hook(output_dir, device_ids):
            import jax

            jax.devices()
            if device_ids:
                ids = (ctypes.c_int64 * len(device_ids))(*device_ids)
                rc = lib.axon_start_nrt_profile(ids, len(device_ids))
            else:
                rc = lib.axon_start_nrt_profile(None, 0)
            if rc != 0:
                raise RuntimeError(f"axon_start_nrt_profile rc={rc}")
            try:
                yield
            finally:
                n = lib.axon_stop_nrt_profile(str(output_dir).encode())
                print(f"ntff profile: {n} file(s) written to {output_dir}")

        mod = types.ModuleType("antenv.axon_hooks")
        mod.get_axon_ntff_profile_hook = lambda: _hook
        mod.set_axon_ntff_profile_hook = lambda h: None
        sys.modules["antenv.axon_hooks"] = mod
    except Exception:
        pass


def _emit(ctx, tc, qt, kt, vp, qk0a, qk0b, ot, mm_dt, scale):
    nc = tc.nc
    inp = ctx.enter_context(tc.tile_pool(name="inp", bufs=3))
    pts = ctx.enter_context(tc.tile_pool(name="pts", bufs=9))
    pti = ctx.enter_context(tc.tile_pool(name="pti", bufs=7))
    outp = ctx.enter_context(tc.tile_pool(name="outp", bufs=4))
    spsum = ctx.enter_context(tc.tile_pool(name="spsum", bufs=3, space="PSUM"))
    opsum = ctx.enter_context(tc.tile_pool(name="opsum", bufs=2, space="PSUM"))

    a_dve = float(1024.0 * np.log2(np.e) * scale)
    b_dve = float(15360.0 - _SCHR_OFF)

    def emit_loads(h):
        """Input DMA loads for head h. kt before qt (LDWEIGHTS consumes kt
        first); vp on the gpsimd queue — it is only needed by the O phase and
        must not delay the S loads."""
        if h == 0:
            # Head 0: K^T/Q^T arrive as host-packed DMAs split across
            # queues so the first S group's operands land with parallel
            # transfers — the head is DMA-latency bound. (Only SP issues
            # here: the Act ring is blocked by ACT_TABLE_LOAD until ~9us
            # and DVE cannot issue DMAs.)
            qk_a = inp.tile([128, 768], mm_dt, tag="qk0a", name="qk0a")
            nc.sync.dma_start(qk_a[:], qk0a[0])
            qk_b = inp.tile([128, 768], mm_dt, tag="qk0b", name="qk0b")
            # kt pairs 2-3 ride the fast SP ring right behind qk0a (unit 0's
            # g2/g3 are gated on them); qt q-half 1 is only needed by unit 1
            # and can take the slower SWDGE ring.
            nc.sync.dma_start(qk_b[:, 0:256], qk0b[0, :, 0:256])
            nc.gpsimd.dma_start(qk_b[:, 256:768], qk0b[0, :, 256:768])

            def kt_at(rows, jp):
                if jp < 2:
                    return qk_a[rows, jp * 128 : (jp + 1) * 128]
                return qk_b[rows, (jp - 2) * 128 : (jp - 1) * 128]

            def qt_at(rows, qh):
                src_t = qk_a if qh == 0 else qk_b
                return src_t[rows, 256:768]
        else:
            kt_t = inp.tile([128, JP * 128], mm_dt, tag="kt", name=f"kt{h}")
            nc.sync.dma_start(kt_t[:], kt[h])
            qt_t = inp.tile([128, N], mm_dt, tag="qt", name=f"qt{h}")
            nc.sync.dma_start(qt_t[:, 0:512], qt[h, :, 0:512])
            nc.sync.dma_start(qt_t[:, 512:1024], qt[h, :, 512:1024])

            def kt_at(rows, jp):
                return kt_t[rows, jp * 128 : (jp + 1) * 128]

            def qt_at(rows, qh):
                return qt_t[rows, qh * 512 : (qh + 1) * 512]

        vp_t = inp.tile([128, KC * 65], mm_dt, tag="vp", name=f"vp{h}")
        nc.gpsimd.dma_start(vp_t[:], vp[h])
        return kt_at, qt_at, vp_t

    def emit_o_mms(h, qh, vp_t, chunks, po):
        """O^T accumulation matmuls for a subset of k-chunks of one unit.
        O^T[m, q] = sum_k (V|1)[k, m] * P^T[k, q]; row 64 is the softmax
        denominator. `chunks` is [(kc, pt, off)] — each chunk's exp lives
        at column offset `off` of (possibly shared, cross-unit) pt tiles."""
        for kc, pt, off in chunks:
            rhs = pt[:, off : off + 512]
            if rhs.dtype == mybir.dt.int16:
                rhs = rhs.bitcast(mybir.dt.float16)
            nc.tensor.matmul(
                po[:],
                lhsT=vp_t[:, kc * 65 : (kc + 1) * 65],
                rhs=rhs,
                start=(kc == 0),
                stop=(kc == KC - 1),
            )

    def emit_o_out(h, qh, po, copy_eng="v", split_store=False):
        """PSUM->SBUF copy-out + store for one finished O^T accumulation."""
        qs = slice(qh * 512, (qh + 1) * 512)
        o_sb = outp.tile([65, 512], mybir.dt.float32, tag="osb", name=f"ou{h}_{qh}")
        if copy_eng == "v":
            nc.vector.tensor_copy(o_sb[:], po[:])
        elif copy_eng == "split":
            nc.vector.tensor_copy(o_sb[:, 0:256], po[:, 0:256])
            nc.scalar.copy(o_sb[:, 256:512], po[:, 256:512])
        else:
            nc.scalar.copy(o_sb[:], po[:])
        if split_store:
            # Two parallel half-row stores shorten the tail-exposed transfer.
            # Both go on hardware-DGE rings (SP + Act): the gpsimd queue is
            # SWDGE, whose slower completion gated the kernel's teardown.
            nc.sync.dma_start(ot[h, 0:33, qs], o_sb[0:33, :])
            nc.scalar.dma_start(ot[h, 33:65, qs], o_sb[33:65, :])
        else:
            nc.sync.dma_start(ot[h, :, qs], o_sb[:])

    def emit_warmup(n_mm=42):
        """HAM pre-warm: the PE clock-gate defaults to half rate and only
        lifts after ~3.4us of SUSTAINED activity; any idle window resets it
        (in-order PE dispatch makes mid-stream gap-filling impossible — a
        fill emitted after a gated matmul inherits its gate; measured).
        Fill the initial DMA-wait gap with tiny zero x zero matmuls into a
        scratch PSUM slot."""
        wz = inp.tile([128, 64], mm_dt, tag="warm", name="warm0")
        nc.gpsimd.memset(wz[:], 0)
        scr = opsum.tile([65, 512], mybir.dt.float32, tag="po", name="warmps")
        for _ in range(n_mm):
            nc.tensor.matmul(
                scr[0:64, 0:64], lhsT=wz[0:64, :], rhs=wz[0:64, :],
                start=True, stop=True,
            )

    def emit_s_group(h, qh, gi, kt_at, qt_at, kc0, glen, eng):
        """S^T matmuls + exp for one k-chunk group of one (head, q-half).
        Even kc use array rows 0-63, odd kc rows 64-127 (tile_position
        auto-derived from base partitions), so each aligned even/odd pair
        of matmuls runs concurrently on the PE. `eng` picks the exp engine:
        'sc' = ScalarE exact exp, 'dve' = VectorE Schraudolph fp16."""
        ps = spsum.tile(
            [128, glen * 512], mybir.dt.float32, tag="ps", name=f"ps{h}_{qh}_{gi}"
        )
        # NOTE: a shared full-array LDWEIGHTS + two non-self-loading matmuls
        # (mm.ldweights = False) is numerically correct but measured ~9us
        # SLOWER: the [128,128] load conflicts with every quadrant, so it
        # cannot be pulled into the background weight buffer while the
        # previous pair streams. Per-matmul half-array loads overlap better.
        for c in range(glen):
            kc = kc0 + c
            jp, half = divmod(kc, 2)
            rows = slice(half * 64, half * 64 + 64)
            nc.tensor.matmul(
                ps[:, c * 512 : (c + 1) * 512],
                lhsT=kt_at(rows, jp),
                rhs=qt_at(rows, qh),
                start=True,
                stop=True,
            )
        if eng == "sc":
            pt = pts.tile([128, glen * 512], mm_dt, tag="pt", name=f"pt{h}_{qh}_{gi}")
            nc.scalar.activation(
                pt[:], ps[:], mybir.ActivationFunctionType.Exp, scale=scale
            )
        else:
            pt = pti.tile(
                [128, glen * 512], mybir.dt.int16, tag="pti", name=f"pi{h}_{qh}_{gi}"
            )
            nc.vector.tensor_scalar(
                pt[:], ps[:], a_dve, b_dve,
                mybir.AluOpType.mult, mybir.AluOpType.add,
            )
        return [(kc0 + c, pt, c * 512) for c in range(glen)]

    # Software pipeline over (head, q-half) units with a skew of 2. Unit
    # i-2's O matmuls are emitted in two 4-chunk batches woven between unit
    # i's S groups, so the PE never sits on a full-PSUM wait with O work
    # stuck behind it in queue order, and consumers get S tiles early.
    # The very first unit splits its first pair-group so the first exp fires
    # after a single matmul — ScalarE's busy span starts earlier.
    # The last iteration drains all remaining O work as early as its exp
    # dependencies allow, leaving only 2 matmuls + copy + store after the
    # final exp.
    first_groups = [(0, 1, "sc"), (1, 1, "sc"), (2, 2, "dve"), (4, 2, "sc"),
                    (6, 2, "dve")]
    units = [(h, qh) for h in range(HPC) for qh in range(QH)]
    tiles = {}
    inflight = []

    def o_tile(h, qh):
        return opsum.tile([65, 512], mybir.dt.float32, tag="po", name=f"po{h}_{qh}")

    for i, (h, qh) in enumerate(units):
        last_it = i == len(units) - 1
        if qh == 0:
            if h == 0:
                emit_warmup()
            tiles[h] = emit_loads(h)
        kt_at, qt_at, vp_t = tiles[h]
        if i == 0:
            groups = first_groups
        else:
            pat = _PAT_A if i % 2 == 0 else _PAT_B
            groups = [(g * 2, 2, pat[g]) for g in range(4)]
        pend = inflight.pop(0) if len(inflight) > 1 else None
        if pend is not None:
            ph, pqh, pvp, pch = pend
            po_p = o_tile(ph, pqh)
        chunks = []
        done_o = 0
        for gi, (kc0, glen, eng) in enumerate(groups):
            if last_it and gi == len(groups) - 1:
                # Drain: all remaining O work is emitted as early as its exp
                # dependencies allow, leaving only 2 matmuls + copy + store
                # after the final exp.
                if pend is not None:
                    emit_o_mms(ph, pqh, pvp, pch[done_o:8], po_p)
                    emit_o_out(ph, pqh, po_p)
                h2, qh2, vp2, ch2 = inflight.pop(0)
                po2 = o_tile(h2, qh2)
                emit_o_mms(h2, qh2, vp2, ch2, po2)
                emit_o_out(h2, qh2, po2, split_store=True)
                po3 = o_tile(h, qh)
                emit_o_mms(h, qh, vp_t, chunks[0:6], po3)
                chunks += emit_s_group(h, qh, gi, kt_at, qt_at, kc0, glen, eng)
                emit_o_mms(h, qh, vp_t, chunks[6:8], po3)
                emit_o_out(h, qh, po3, copy_eng="split", split_store=True)
                break
            chunks += emit_s_group(h, qh, gi, kt_at, qt_at, kc0, glen, eng)
            if pend is not None and gi in (1, 2):
                # Two 4-chunk O batches woven between S groups: keeps the PE
                # queue free of full-PSUM waits without paying too many
                # S<->O accumulation-context switches (2-chunk batches after
                # every group measured slower). The copy-out goes right after
                # the last batch — before the final S group — so the opsum
                # slot frees a pair-slot earlier and the DVE copy overlaps g3.
                emit_o_mms(ph, pqh, pvp, pch[done_o : done_o + 4], po_p)
                done_o += 4
                if gi == 2:
                    emit_o_out(ph, pqh, po_p)
                    pend = None
        if not last_it:
            inflight.append((h, qh, vp_t, chunks))


def _build(mm_dt, scale):
    nc = bacc.Bacc(
        "TRN2",
        target_bir_lowering=False,
        debug=False,
        enable_asserts=False,
        num_devices=NCORES,
    )
    qt_d = nc.dram_tensor("qt", [HPC, 128, N], mm_dt, kind="ExternalInput")
    kt_d = nc.dram_tensor("kt", [HPC, 128, JP * 128], mm_dt, kind="ExternalInput")
    vp_d = nc.dram_tensor("vp", [HPC, 128, KC * 65], mm_dt, kind="ExternalInput")
    qk0a_d = nc.dram_tensor("qk0a", [1, 128, 768], mm_dt, kind="ExternalInput")
    qk0b_d = nc.dram_tensor("qk0b", [1, 128, 768], mm_dt, kind="ExternalInput")
    ot_d = nc.dram_tensor("ot", [HPC, 65, N], mybir.dt.float32, kind="ExternalOutput")
    with tile.TileContext(nc) as tc:
        with ExitStack() as ctx:
            _emit(ctx, tc, qt_d.ap(), kt_d.ap(), vp_d.ap(), qk0a_d.ap(), qk0b_d.ap(), ot_d.ap(), mm_dt, scale)
    nc.compile()
    return nc


def _get_nc(mm_dt, scale):
    key = (mm_dt, scale)
    if key not in _NC_CACHE:
        _NC_CACHE[key] = _build(mm_dt, scale)
    return _NC_CACHE[key]


def kernel(Q, K, V, qkv=None, **_unused):
    global LAST_RESULTS
    Q = np.asarray(Q, dtype=np.float32)
    K = np.asarray(K, dtype=np.float32)
    V = np.asarray(V, dtype=np.float32)

    # Host-side layout prep (not part of HW exec time).
    Qt = Q.transpose(0, 1, 3, 2)                       # [B, H, D, N]
    QtD = np.concatenate([Qt, Qt], axis=2)             # [B, H, 128, N]
    Kt = K.transpose(0, 1, 3, 2)                       # [B, H, D, N]
    KtP = (
        Kt.reshape(B, H, D, JP, 2, 128)
        .transpose(0, 1, 4, 2, 3, 5)
        .reshape(B, H, 128, JP * 128)
    )
    Vp = np.ones((B, H, 128, KC * 65), dtype=np.float32)
    Vp.reshape(B, H, 128, KC, 65)[..., :64] = V.reshape(B, H, KC, 128, D).transpose(
        0, 1, 3, 2, 4
    )

    QtD = QtD.astype(np.float16)
    KtP = KtP.astype(np.float16)
    Vp = Vp.astype(np.float16)

    trace = bool(int(os.environ.get("ATT_TRACE", "0")))
    if trace:
        _install_ntff_hook()
    scale = 1.0 / float(np.sqrt(np.float64(int(qkv)))) if qkv is not None else (
        1.0 / float(np.sqrt(np.float64(D)))
    )
    nc = _get_nc(_MM_DT, scale)
    in_maps = [
        {
            "qt": np.ascontiguousarray(QtD[c]),
            "kt": np.ascontiguousarray(KtP[c]),
            "vp": np.ascontiguousarray(Vp[c]),
            "qk0a": np.ascontiguousarray(
                np.concatenate(
                    [KtP[c, 0, :, 0:256], QtD[c, 0, :, 0:512]], axis=-1
                )[None]
            ),
            "qk0b": np.ascontiguousarray(
                np.concatenate(
                    [KtP[c, 0, :, 256:512], QtD[c, 0, :, 512:1024]], axis=-1
                )[None]
            ),
        }
        for c in range(NCORES)
    ]
    res = run_bass_kernel_spmd(
        nc,
        in_maps,
        core_ids=list(range(NCORES)),
        trace=trace,
    )
    LAST_RESULTS = res

    out = np.empty((B, H, N, D), dtype=np.float32)
    for c in range(NCORES):
        ot = res.results[c]["ot"]                      # [HPC, 65, N]
        denom = ot[:, 64:65, :]                        # [HPC, 1, N]
        out[c] = (ot[:, :64, :] / denom).transpose(0, 2, 1)
    return out
